# revision 9
# baseline (speedup 1.0000x reference)
"""Multi-head attention TRN2 kernel (nn_Attention_48859547959768).

Head-parallel tensor parallelism across 8 NeuronCores: each core computes
2 of the 16 heads end-to-end (column-parallel QKV projection, attention,
row-parallel output projection) and returns a partial [B,S,DIM] output;
the host sums the 8 partials and adds the output bias.

Per-core dataflow:
  - X^T staged in SBUF per batch; Q^T/K^T/V^T = W^T @ X^T + b (PE).
  - scores^T[k,q] = K'^T.T @ Q'^T with a 65th contraction row carrying the
    additive mask penalty (K side) against ones (Q side).
  - exp on ScalarE with the 1/sqrt(D) scale folded into the activation's
    free affine; no row-max subtraction (scores are O(1) here).
  - ctx'^T[dv,q] = V'.T @ exp^T with a 65th V column of ones so row 64
    accumulates the softmax denominators for free.
  - per-q normalization via a PE-transpose "dance" (transpose 128-chunks,
    reciprocal + per-partition scale, transpose back).
  - out_partial[t,:] = ctx^T.T @ Wo_slice, streamed to DRAM.

Matmul dtypes are configurable per stage: fp32r (fp32 bits, single
HIGH-mode pass, ~2 cyc/row) or bf16 (1 cyc/row, ~2x faster, ~2e-3 err).
"""

import numpy as np

B, S, DIM = 2, 2048, 1024
H, D = 16, 64
N_CORES = 8
HPC = H // N_CORES  # heads per core = 2
DHC = HPC * D       # per-core head-dim slice = 128
KT = DIM // 128     # contraction tiles for projections = 8
NSPAN = S // 512    # q spans = 4
NCHUNK = S // 128   # 128-token chunks = 16

ATTN_BF16 = True    # scores + PV matmuls in bf16
PROJ_BF16 = False   # QKV projections in bf16
OUT_BF16 = False    # output projection in bf16

_cached = {}


def _build():
    import concourse.mybir as mybir
    from concourse import bacc
    from concourse.masks import make_identity
    from concourse.tile import TileContext

    f32 = mybir.dt.float32
    f32r = mybir.dt.float32r
    bf16 = mybir.dt.bfloat16
    dt_at = bf16 if ATTN_BF16 else f32r
    dt_pr = bf16 if PROJ_BF16 else f32r
    dt_ou = bf16 if OUT_BF16 else f32r
    Exp = mybir.ActivationFunctionType.Exp

    nc = bacc.Bacc("TRN2", target_bir_lowering=False)

    xt = nc.dram_tensor("xt", [B, DIM, S], dt_pr, kind="ExternalInput").ap()
    pen = nc.dram_tensor("pen", [B, S], dt_at, kind="ExternalInput").ap()
    wq = nc.dram_tensor("wq", [DIM, DHC], dt_pr, kind="ExternalInput").ap()
    wk = nc.dram_tensor("wk", [DIM, DHC], dt_pr, kind="ExternalInput").ap()
    wv = nc.dram_tensor("wv", [DIM, DHC], dt_pr, kind="ExternalInput").ap()
    wo = nc.dram_tensor("wo", [DHC, DIM], dt_ou, kind="ExternalInput").ap()
    bqd = nc.dram_tensor("bq", [DHC, 1], f32, kind="ExternalInput").ap()
    bkd = nc.dram_tensor("bk", [DHC, 1], f32, kind="ExternalInput").ap()
    bvd = nc.dram_tensor("bv", [DHC, 1], f32, kind="ExternalInput").ap()
    out = nc.dram_tensor("out", [B, S, DIM], f32, kind="ExternalOutput").ap()

    with TileContext(nc) as tc:
        from contextlib import ExitStack

        with ExitStack() as ctx:
            const = ctx.enter_context(tc.tile_pool(name="const", bufs=1))
            xtp = ctx.enter_context(tc.tile_pool(name="xtp", bufs=KT))
            persist = ctx.enter_context(tc.tile_pool(name="persist", bufs=1))
            ctxp = ctx.enter_context(tc.tile_pool(name="ctxp", bufs=2))
            work = ctx.enter_context(tc.tile_pool(name="work", bufs=3))
            ps_sc = ctx.enter_context(tc.tile_pool(name="ps_sc", bufs=3, space="PSUM"))
            ps_pc = ctx.enter_context(tc.tile_pool(name="ps_pc", bufs=2, space="PSUM"))
            ps_po = ctx.enter_context(tc.tile_pool(name="ps_po", bufs=3, space="PSUM"))

            ident = const.tile([128, 128], dt_at)
            make_identity(nc, ident)
            wq_sb = const.tile([128, KT, DHC], dt_pr)
            wk_sb = const.tile([128, KT, DHC], dt_pr)
            wv_sb = const.tile([128, KT, DHC], dt_pr)
            nc.sync.dma_start(out=wq_sb, in_=wq.rearrange("(kt p) m -> p kt m", p=128))
            nc.sync.dma_start(out=wk_sb, in_=wk.rearrange("(kt p) m -> p kt m", p=128))
            nc.sync.dma_start(out=wv_sb, in_=wv.rearrange("(kt p) m -> p kt m", p=128))
            wo_sb = const.tile([128, DIM], dt_ou)
            nc.sync.dma_start(out=wo_sb, in_=wo)
            bq_sb = const.tile([128, 1], f32)
            bk_sb = const.tile([128, 1], f32)
            bv_sb = const.tile([128, 1], f32)
            nc.sync.dma_start(out=bq_sb, in_=bqd)
            nc.sync.dma_start(out=bk_sb, in_=bkd)
            nc.sync.dma_start(out=bv_sb, in_=bvd)

            for b in range(B):
                # ---- stage X^T for this batch (2 DMAs per k-tile) ----
                with nc.named_scope(f"load{b}"):
                    xt_t = []
                    for kt in range(KT):
                        t = xtp.tile([128, S], dt_pr, tag="xt", name=f"xt{kt}")
                        half = S // 2
                        for hh in range(2):
                            nc.sync.dma_start(
                                out=t[:, hh * half:(hh + 1) * half],
                                in_=xt[b, kt * 128:(kt + 1) * 128,
                                       hh * half:(hh + 1) * half])
                        xt_t.append(t)

                # ---- projections: Q^T, K^T, V^T = W.T @ X^T + bias ----
                qtp = [persist.tile([65, S], dt_at, tag=f"qtp{h}", name=f"qtp{h}")
                       for h in range(HPC)]
                ktp = [persist.tile([65, S], dt_at, tag=f"ktp{h}", name=f"ktp{h}")
                       for h in range(HPC)]
                vt = persist.tile([128, S], dt_at, tag="vt")

                def project(w_sb, bias_sb, dests, dest_slices):
                    for sp in range(NSPAN):
                        ps = ps_po.tile([128, 512], f32, tag="po", name="ps")
                        for kt in range(KT):
                            nc.tensor.matmul(
                                ps,
                                w_sb[:, kt, :],
                                xt_t[kt][:, sp * 512:(sp + 1) * 512],
                                start=(kt == 0),
                                stop=(kt == KT - 1),
                            )
                        for h in range(HPC):
                            dst, rows = dests[h], dest_slices[h]
                            nc.vector.tensor_scalar_add(
                                out=dst[rows, sp * 512:(sp + 1) * 512],
                                in0=ps[h * 64:(h + 1) * 64, :],
                                scalar1=bias_sb[h * 64:(h + 1) * 64, 0:1],
                            )

                with nc.named_scope(f"proj{b}"):
                    project(wq_sb, bq_sb, qtp, [slice(0, 64)] * HPC)
                    project(wk_sb, bk_sb, ktp, [slice(0, 64)] * HPC)
                    project(wv_sb, bv_sb, [vt] * HPC,
                            [slice(h * 64, (h + 1) * 64) for h in range(HPC)])

                    for h in range(HPC):
                        nc.vector.memset(qtp[h][64:65, :], 1.0)
                        nc.sync.dma_start(out=ktp[h][64:65, :], in_=pen[b:b + 1, :])

                    # ---- V' = V (tokens on partitions) with ones column ----
                    vp = [persist.tile([128, NCHUNK, 65], dt_at,
                                       tag=f"vp{h}", name=f"vp{h}")
                          for h in range(HPC)]
                    for h in range(HPC):
                        nc.vector.memset(vp[h][:, :, 64:65], 1.0)
                    for c in range(NCHUNK):
                        pt = ps_po.tile([128, 512], dt_at, tag="po", name="pt")
                        nc.tensor.transpose(
                            pt[:, 0:128], vt[:, c * 128:(c + 1) * 128], ident)
                        for h in range(HPC):
                            nc.vector.tensor_copy(
                                out=vp[h][:, c, 0:64],
                                in_=pt[:, h * 64:(h + 1) * 64])

                # ---- attention per head ----
                ctxt = [ctxp.tile([65, S], f32, tag="ctxt", name=f"ctxt{h}")
                        for h in range(HPC)]
                for h in range(HPC):
                    with nc.named_scope(f"attn{b}_{h}"):
                        for sp in range(NSPAN):
                            pc = ps_pc.tile([65, 512], f32, tag="pc", name="pc")
                            prev = None
                            for kt in range(NCHUNK):
                                sc = ps_sc.tile([128, 512], f32, tag="sc", name="sc")
                                nc.tensor.matmul(
                                    sc,
                                    ktp[h][:, kt * 128:(kt + 1) * 128],
                                    qtp[h][:, sp * 512:(sp + 1) * 512],
                                    start=True, stop=True,
                                )
                                if prev is not None:
                                    pkt, pe = prev
                                    nc.tensor.matmul(
                                        pc, vp[h][:, pkt, :], pe,
                                        start=(pkt == 0), stop=False)
                                e = work.tile([128, 512], dt_at, tag="expT", name="e")
                                nc.scalar.activation(e, sc, Exp, scale=0.125)
                                prev = (kt, e)
                            pkt, pe = prev
                            nc.tensor.matmul(pc, vp[h][:, pkt, :], pe,
                                             start=False, stop=True)
                            nc.vector.tensor_copy(
                                out=ctxt[h][:, sp * 512:(sp + 1) * 512], in_=pc)

                # ---- normalization (DMA spread/broadcast, no PE) ----
                ctxtn = persist.tile([128, S], dt_ou, tag="ctxtn")
                for h in range(HPC):
                    with nc.named_scope(f"norm{b}_{h}"):
                        rtmp = work.tile([128, S // 128], f32, tag="rtmp",
                                         name="rtmp")
                        nc.sync.dma_start(
                            out=rtmp,
                            in_=ctxt[h][64:65, :].rearrange(
                                "o (p c) -> o p c", p=128))
                        rec = work.tile([128, S // 128], f32, tag="rec",
                                        name="rec")
                        nc.vector.reciprocal(rec, rtmp)
                        rrow = work.tile([1, S], f32, tag="rrow", name="rrow")
                        nc.sync.dma_start(
                            out=rrow.rearrange("o (p c) -> o p c", p=128),
                            in_=rec)
                        rt = work.tile([64, S], f32, tag="rt", name="rt")
                        nc.gpsimd.partition_broadcast(out_ap=rt, in_ap=rrow)
                        nc.vector.tensor_mul(
                            out=ctxtn[h * 64:(h + 1) * 64, :],
                            in0=ctxt[h][0:64, :], in1=rt)

                with nc.named_scope(f"outproj{b}"):
                    for c in range(NCHUNK):
                        ob = work.tile([128, DIM], f32, tag="ob", name="ob")
                        for osp in range(2):
                            po = ps_po.tile([128, 512], f32, tag="po", name="po")
                            nc.tensor.matmul(
                                po,
                                ctxtn[:, c * 128:(c + 1) * 128],
                                wo_sb[:, osp * 512:(osp + 1) * 512],
                                start=True, stop=True,
                            )
                            nc.vector.tensor_copy(
                                out=ob[:, osp * 512:(osp + 1) * 512], in_=po)
                        nc.sync.dma_start(
                            out=out[b, c * 128:(c + 1) * 128, :], in_=ob)

    nc.compile()
    return nc


def _get_nc():
    if "nc" not in _cached:
        _cached["nc"] = _build()
    return _cached["nc"]


def _np_dt(bf):
    if bf:
        import ml_dtypes
        return ml_dtypes.bfloat16
    return np.float32


def kernel(X, mask, Wq, bq, Wk, bk, Wv, bv, Wo, bo):
    from concourse.bass_utils import run_bass_kernel_spmd

    X = np.asarray(X, dtype=np.float32)
    mask = np.asarray(mask, dtype=np.float32)
    Wq, Wk, Wv, Wo = (np.asarray(a, dtype=np.float32) for a in (Wq, Wk, Wv, Wo))
    bq, bk, bv, bo = (np.asarray(a, dtype=np.float32) for a in (bq, bk, bv, bo))

    tp = _np_dt(PROJ_BF16)
    ta = _np_dt(ATTN_BF16)
    to = _np_dt(OUT_BF16)
    xtf = np.ascontiguousarray(X.transpose(0, 2, 1)).astype(tp)  # [B, DIM, S]
    penf = (-8e6 * (1.0 - mask)).astype(ta)

    in_maps = []
    for c in range(N_CORES):
        sl = slice(c * DHC, (c + 1) * DHC)
        in_maps.append({
            "xt": xtf,
            "pen": penf,
            "wq": np.ascontiguousarray(Wq[:, sl]).astype(tp),
            "wk": np.ascontiguousarray(Wk[:, sl]).astype(tp),
            "wv": np.ascontiguousarray(Wv[:, sl]).astype(tp),
            "wo": np.ascontiguousarray(Wo[sl, :]).astype(to),
            "bq": np.ascontiguousarray(bq[sl].reshape(DHC, 1)),
            "bk": np.ascontiguousarray(bk[sl].reshape(DHC, 1)),
            "bv": np.ascontiguousarray(bv[sl].reshape(DHC, 1)),
        })

    res = run_bass_kernel_spmd(_get_nc(), in_maps, core_ids=list(range(N_CORES)))
    _cached["last_results"] = res
    acc = res.results[0]["out"].astype(np.float32).copy()
    for c in range(1, N_CORES):
        acc += res.results[c]["out"]
    acc += bo[None, None, :]
    return acc.astype(np.float32)
